# revision 1
# baseline (speedup 1.0000x reference)
"""Trainium2 Bass kernel for nn_Attention_21303037788751 (sparse_attention).

Reference computation (B=16, N=512, F=256, H=8, D=64):
    qkv  = node @ W_qkv                      -> q, k, v  [B,H,N,D]
    attn = softmax(q k^T / sqrt(D)) + 0.5*adj + 0.5*exp(-dist)
    out  = (attn @ v) reshaped  @ W_out + b_out

Sharding: data-parallel over batch, 2 batches per core on 8 NeuronCores.

Per-core program (col-major formulation; all matmuls float32r):
    nodeT = node^T (PE transpose)            [F, N]
    qT,kT per head-pair = W_qk^T @ nodeT     [128=(2 heads), N]
    v     = node @ W_v (row-major)           [N, H*D]
    ST_h  = kT_h-slices ^T @ qT_h            [N_j, N_i]  (K=64, head pairs on
                                              disjoint PE row strips)
    E_h   = exp(ST_h / 8)  (ACT, PSUM->SBUF) -- logits are tiny, no max pass
    OT1_h = vaug_h^T @ E_h: the stationary operand is V_h augmented with a
            ones column (odd heads also get 63 leading zero columns), so one
            accumulation group yields both V^T E and the softmax column sums,
            with odd heads landing at partitions 64..127.
    softmax part = OT1_h * bcast(1/sums)      (DVE recip + DMA bcast)
    G     = adj + exp(-dist); GT by PE transpose
    OT2_p = [v_e|v_o]^T @ GT                  [128, N_i]
    Y     = softmaxT^T @ W_out + OT2^T @ (0.5*W_out) + ones^T @ b_out
"""

import sys

sys.path.insert(0, "/opt/trn_rl_repo")

import numpy as np

B, N, F = 16, 512, 256
H, D = 8, 64
INNER = H * D          # 512
NC_COUNT = 8
PB = B // NC_COUNT     # batches per core
P = 128
SCALE = D ** -0.5      # 0.125
VBLK = 193             # per-pair vaug block: [1 | 0*63 | v_o(64) | v_e(64) | 1]

_CACHE = {}


def _col_perm():
    """Column permutation of W_qkv: head-pair [q_h0|q_h1|k_h0|k_h1] blocks,
    then all v columns grouped by head."""
    order = []
    for p in range(H // 2):
        h0, h1 = 2 * p, 2 * p + 1
        order += [h0 * 192 + d for d in range(64)]
        order += [h1 * 192 + d for d in range(64)]
        order += [h0 * 192 + 64 + d for d in range(64)]
        order += [h1 * 192 + 64 + d for d in range(64)]
    for h in range(H):
        order += [h * 192 + 128 + d for d in range(64)]
    return np.array(order)


def build_program():
    import concourse.bass as bass
    import concourse.tile as tile
    from concourse import bacc, mybir
    from concourse.masks import make_identity

    f32 = mybir.dt.float32
    f32r = mybir.dt.float32r

    nc = bacc.Bacc("TRN2", target_bir_lowering=False, debug=False,
                   num_devices=NC_COUNT)

    node_d = nc.dram_tensor("node", [PB, N, F], f32, kind="ExternalInput").ap()
    adj_d = nc.dram_tensor("adj", [PB, N, N], f32, kind="ExternalInput").ap()
    dist_d = nc.dram_tensor("dist", [PB, N, N], f32, kind="ExternalInput").ap()
    wqkv_d = nc.dram_tensor("wqkv", [F, 3 * INNER], f32, kind="ExternalInput").ap()
    wout_d = nc.dram_tensor("wout", [INNER, F], f32, kind="ExternalInput").ap()
    wouth_d = nc.dram_tensor("wouth", [INNER, F], f32, kind="ExternalInput").ap()
    bout_d = nc.dram_tensor("bout", [1, F], f32, kind="ExternalInput").ap()
    out_d = nc.dram_tensor("out", [PB, N, F], f32, kind="ExternalOutput").ap()

    with tile.TileContext(nc) as tc:
        with tc.tile_pool(name="const", bufs=1) as cpool, \
             tc.tile_pool(name="stage", bufs=2) as spool, \
             tc.tile_pool(name="work2", bufs=2) as wpool, \
             tc.tile_pool(name="qk", bufs=8) as qkpool, \
             tc.tile_pool(name="ypool", bufs=4) as ypool, \
             tc.tile_pool(name="dx", bufs=3) as dxpool, \
             tc.tile_pool(name="ps2", bufs=2, space="PSUM") as ps2, \
             tc.tile_pool(name="ps1", bufs=4, space="PSUM") as ps1:

            S = [dict() for _ in range(PB)]

            # ---- stage inputs (node first; adj/dist b0 next) ---------------
            for b in range(PB):
                s = S[b]
                s["node"] = wpool.tile([P, 4, F], f32, tag="node",
                                       name=f"node_{b}")
                nc.sync.dma_start(s["node"][:],
                                  node_d[b].rearrange("(p nb) f -> p nb f", nb=4))
            for b in range(PB):
                s = S[b]
                s["adj"] = wpool.tile([P, 4, N], f32, tag="adj", name=f"adj_{b}")
                s["dist"] = dxpool.tile([P, 4, N], f32, tag="dx",
                                        name=f"dist_{b}")
            nc.sync.dma_start(S[0]["adj"][:],
                              adj_d[0].rearrange("(p ib) j -> p ib j", ib=4))
            nc.sync.dma_start(S[0]["dist"][:],
                              dist_d[0].rearrange("(p ib) j -> p ib j", ib=4))

            # ---- constants -------------------------------------------------
            ident = cpool.tile([P, P], f32)
            make_identity(nc, ident)

            wqkv_r = cpool.tile([P, 2, 3 * INNER], f32r)
            wq_view = wqkv_d.rearrange("(kt p) m -> p kt m", p=P)
            # v columns (chunk 2) are consumed first
            for ch in (2, 0, 1):
                cs = slice(ch * INNER, (ch + 1) * INNER)
                wq_st = spool.tile([P, 2, INNER], f32, tag="stg",
                                   name=f"wq_st_{ch}")
                nc.sync.dma_start(wq_st[:], wq_view[:, :, cs])
                nc.vector.tensor_copy(wqkv_r[:, :, cs], wq_st[:])

            wout_st = spool.tile([P, 4, F], f32, tag="stg", name="wout_st")
            nc.sync.dma_start(wout_st[:], wout_d.rearrange("(kt p) f -> p kt f", p=P))
            wout_r = cpool.tile([P, 4, F], f32r)
            nc.vector.tensor_copy(wout_r[:], wout_st[:])
            wouth_st = spool.tile([P, 4, F], f32, tag="stg", name="wouth_st")
            nc.sync.dma_start(wouth_st[:], wouth_d.rearrange("(kt p) f -> p kt f", p=P))
            wouth_r = cpool.tile([P, 4, F], f32r)
            nc.vector.tensor_copy(wouth_r[:], wouth_st[:])

            bout_st = spool.tile([1, F], f32, tag="stg2")
            nc.sync.dma_start(bout_st[:], bout_d[:])
            bout_r = cpool.tile([1, F], f32r)
            nc.vector.tensor_copy(bout_r[:], bout_st[:])

            ones_st = cpool.tile([1, P], f32)
            nc.vector.memset(ones_st[:], 1.0)
            ones_row = cpool.tile([1, P], f32r)
            nc.vector.tensor_copy(ones_row[:], ones_st[:])

            # vaug pad pattern: block head [1, 0*63] (odd-head ones + zeros),
            # block tail col 192 is the even-head ones column
            pat_st = cpool.tile([P, 64], f32)
            nc.vector.memset(pat_st[:], 0.0)
            nc.vector.memset(pat_st[:, 0:1], 1.0)
            one_st = cpool.tile([P, 1], f32)
            nc.vector.memset(one_st[:], 1.0)

            nc.sync.dma_start(S[1]["adj"][:],
                              adj_d[1].rearrange("(p ib) j -> p ib j", ib=4))
            nc.sync.dma_start(S[1]["dist"][:],
                              dist_d[1].rearrange("(p ib) j -> p ib j", ib=4))

            # ---- nodeT ----------------------------------------------------
            for b in range(PB):
                s = S[b]
                s["nodeT"] = wpool.tile([P, 2, N], f32r, tag="nodeT",
                                        name=f"nodeT_{b}")
                tr_ps = ps2.tile([P, 2, N], f32, tag="ps2", name=f"ntr_{b}")
                for kt in range(2):
                    for nb in range(4):
                        nc.tensor.transpose(
                            tr_ps[:, kt, nb * P:(nb + 1) * P],
                            s["node"][:, nb, kt * P:(kt + 1) * P],
                            ident[:])
                nc.vector.tensor_copy(s["nodeT"][:], tr_ps[:])

            # ---- v projection into augmented stationary layout ------------
            for b in range(PB):
                s = S[b]
                vaug = wpool.tile([P, 4, 4, VBLK], f32r, tag="v",
                                  name=f"vaug_{b}")
                s["vaug"] = vaug
                nc.scalar.copy(
                    vaug[:, :, :, 0:64],
                    pat_st[:, None, None, :].to_broadcast((P, 4, 4, 64)))
                nc.scalar.copy(
                    vaug[:, :, :, 192:193],
                    one_st[:, None, None, :].to_broadcast((P, 4, 4, 1)))
                for jh in range(2):
                    v_ps = ps2.tile([P, 2, N], f32, tag="ps2",
                                    name=f"vps_{b}_{jh}")
                    for j in range(2):
                        jb = jh * 2 + j
                        for kt in range(2):
                            nc.tensor.matmul(
                                v_ps[:, j, :],
                                s["nodeT"][:, kt, jb * P:(jb + 1) * P],
                                wqkv_r[:, kt, 2 * INNER:3 * INNER],
                                start=(kt == 0), stop=(kt == 1))
                    v4 = v_ps[:].rearrange("q two (pr par d) -> q two pr par d",
                                           par=2, d=64)
                    nc.scalar.copy(vaug[:, jh * 2:jh * 2 + 2, :, 128:192],
                                   v4[:, :, :, 0, :])
                    nc.scalar.copy(vaug[:, jh * 2:jh * 2 + 2, :, 64:128],
                                   v4[:, :, :, 1, :])

            # ---- G = adj + exp(-dist); GT ---------------------------------
            for b in range(PB):
                s = S[b]
                nc.scalar.activation(s["dist"][:], s["dist"][:],
                                     mybir.ActivationFunctionType.Exp,
                                     scale=-1.0)
                g_sb = s["adj"]
                nc.gpsimd.tensor_add(g_sb[:], s["adj"][:], s["dist"][:])
                gt_r = wpool.tile([P, 4, N], f32r, tag="gt", name=f"gt_{b}")
                s["gt"] = gt_r
                g4 = g_sb[:].rearrange("q ib (jq four) -> q ib jq four",
                                       four=4)
                for jh in range(2):
                    tr_ps = ps2.tile([P, 2, N], f32, tag="ps2",
                                     name=f"gtr_{b}_{jh}")
                    for j in range(2):
                        jb = jh * 2 + j
                        for ib in range(4):
                            nc.tensor.transpose(
                                tr_ps[:, j, ib * P:(ib + 1) * P],
                                g4[:, ib, :, jb],
                                ident[:])
                    nc.vector.tensor_copy(gt_r[:, jh * 2:jh * 2 + 2, :],
                                          tr_ps[:])

            # ---- qT/kT projections ----------------------------------------
            for b in range(PB):
                s = S[b]
                s["qk"] = []
                for p in range(H // 2):
                    qk_ps = ps2.tile([P, 2, N], f32, tag="ps2",
                                     name=f"qkps_{b}_{p}")
                    base = p * 256
                    for kt in range(2):
                        nc.tensor.matmul(
                            qk_ps[:, 0, :], wqkv_r[:, kt, base:base + P],
                            s["nodeT"][:, kt, :],
                            start=(kt == 0), stop=(kt == 1))
                    for kt in range(2):
                        nc.tensor.matmul(
                            qk_ps[:, 1, :], wqkv_r[:, kt, base + P:base + 256],
                            s["nodeT"][:, kt, :],
                            start=(kt == 0), stop=(kt == 1))
                    qk = qkpool.tile([P, 2, N], f32r, tag="qk",
                                     name=f"qk_{b}_{p}")
                    nc.vector.tensor_copy(qk[:], qk_ps[:])
                    s["qk"].append(qk)

            # ---- attention: batches interleaved per head ------------------
            for b in range(PB):
                s = S[b]
                s["otfin"] = wpool.tile([P, 4, N], f32r, tag="otfin",
                                        name=f"otfin_{b}")
                s["ot2"] = wpool.tile([P, 4, N], f32r, tag="ot2",
                                      name=f"ot2_{b}")
            for p in range(H // 2):
                for b in range(PB):
                    s = S[b]
                    vaug, gt_r = s["vaug"], s["gt"]
                    qq = s["qk"][p][:, 0, :]
                    kk = s["qk"][p][:, 1, :]
                    ex = [dxpool.tile([P, 4, N], f32r, tag="dx",
                                      name=f"expst_{b}_{2 * p}"),
                          dxpool.tile([P, 4, N], f32r, tag="dx",
                                      name=f"expst_{b}_{2 * p + 1}")]
                    for half in range(2):
                        st = [ps2.tile([P, 2, N], f32, tag="ps2",
                                       name=f"st_e_{b}_{p}_{half}"),
                              ps2.tile([P, 2, N], f32, tag="ps2",
                                       name=f"st_o_{b}_{p}_{half}")]
                        # even/odd heads on disjoint PE row strips -> the
                        # alternating matmuls execute concurrently
                        for j in range(2):
                            jb = half * 2 + j
                            for odd in range(2):
                                lo = odd * 64
                                nc.tensor.matmul(
                                    st[odd][:, j, :],
                                    kk[lo:lo + 64, jb * P:(jb + 1) * P],
                                    qq[lo:lo + 64, :],
                                    start=True, stop=True)
                        for odd in range(2):
                            nc.scalar.activation(
                                ex[odd][:, half * 2:half * 2 + 2, :],
                                st[odd][:],
                                mybir.ActivationFunctionType.Exp, scale=SCALE)

                    for odd in range(2):
                        ot1_ps = ps1.tile([P, N], f32, tag="ps1",
                                          name=f"ot1_{b}_{p}_{odd}")
                        if not odd:
                            out_sl, av_sl, sm_sl = (slice(0, 65), slice(0, 64),
                                                    slice(64, 65))
                            rc_sl = slice(0, 65)
                        else:
                            out_sl, av_sl, sm_sl = (slice(0, P), slice(64, P),
                                                    slice(0, 1))
                            rc_sl = slice(0, 1)
                        for jb in range(4):
                            if not odd:
                                lhsT = vaug[:, jb, p, 128:VBLK]
                            else:
                                lhsT = vaug[:, jb, p, 0:128]
                            nc.tensor.matmul(
                                ot1_ps[out_sl, :], lhsT, ex[odd][:, jb, :],
                                start=(jb == 0), stop=(jb == 3))

                        rec = wpool.tile([P, N], f32, tag="rec")
                        # approx recip is broken at base partition 64; run it
                        # from partition 0 over the span (extra rows unused)
                        nc.vector.reciprocal_approx_fast(rec[rc_sl, :],
                                                         ot1_ps[rc_sl, :])
                        recbc = wpool.tile([P, N], f32, tag="recbc")
                        nc.gpsimd.dma_start(
                            recbc[av_sl, :],
                            rec[sm_sl, None, :].to_broadcast((1, 64, N)))
                        nc.vector.tensor_tensor(
                            s["otfin"][av_sl, p, :], ot1_ps[av_sl, :],
                            recbc[av_sl, :], mybir.AluOpType.mult)

                    # G-part matmul; PE filler while exps/epilogue drain
                    ot2_ps = ps1.tile([P, N], f32, tag="ps1",
                                      name=f"ot2_{b}_{p}")
                    for jb in range(4):
                        nc.tensor.matmul(
                            ot2_ps[:], vaug[:, jb, p, 64:192],
                            gt_r[:, jb, :],
                            start=(jb == 0), stop=(jb == 3))
                    nc.scalar.copy(s["ot2"][:, p, :], ot2_ps[:])

            # ---- output projection ----------------------------------------
            for b in range(PB):
                s = S[b]
                for nb in range(4):
                    y_ps = ps1.tile([P, F], f32, tag="ps1", name=f"y_{b}_{nb}")
                    for kt in range(4):
                        nc.tensor.matmul(
                            y_ps[:], s["otfin"][:, kt, nb * P:(nb + 1) * P],
                            wout_r[:, kt, :],
                            start=(kt == 0), stop=False)
                    for kt in range(4):
                        nc.tensor.matmul(
                            y_ps[:], s["ot2"][:, kt, nb * P:(nb + 1) * P],
                            wouth_r[:, kt, :],
                            start=False, stop=False)
                    nc.tensor.matmul(y_ps[:], ones_row[:], bout_r[:],
                                     start=False, stop=True)
                    y_sb = ypool.tile([P, F], f32, tag="y")
                    nc.vector.tensor_copy(y_sb[:], y_ps[:])
                    nc.sync.dma_start(out_d[b].rearrange("(p four) f -> p four f", four=4)[:, nb, :], y_sb[:])

    nc.compile()
    return nc


def _get_program():
    if "nc" not in _CACHE:
        _CACHE["nc"] = build_program()
    return _CACHE["nc"]


def run(inputs, trace=False):
    """Run on 8 cores; returns (full_output, BassKernelResults)."""
    from concourse.bass_utils import run_bass_kernel_spmd

    nc = _get_program()
    wqkv_p = np.ascontiguousarray(inputs["W_qkv"][:, _col_perm()], dtype=np.float32)
    wout = np.ascontiguousarray(inputs["W_out"], dtype=np.float32)
    swap = np.arange(INNER).reshape(4, 2, 64)[:, ::-1, :].reshape(-1)
    wouth = np.ascontiguousarray(0.5 * wout[swap], dtype=np.float32)
    bout = np.ascontiguousarray(inputs["b_out"], dtype=np.float32).reshape(1, F)

    in_maps = []
    for c in range(NC_COUNT):
        sl = slice(c * PB, (c + 1) * PB)
        in_maps.append({
            "node": np.ascontiguousarray(inputs["node"][sl], dtype=np.float32),
            "adj": np.ascontiguousarray(inputs["adj"][sl], dtype=np.float32),
            "dist": np.ascontiguousarray(inputs["dist"][sl], dtype=np.float32),
            "wqkv": wqkv_p,
            "wout": wout,
            "wouth": wouth,
            "bout": bout,
        })
    res = run_bass_kernel_spmd(nc, in_maps, core_ids=list(range(NC_COUNT)),
                               trace=trace)
    out = np.concatenate([res.results[c]["out"] for c in range(NC_COUNT)], axis=0)
    return out, res


def kernel(node, adj, dist, node_mask, adj_mask, dist_mask, W_qkv, W_out, b_out):
    inputs = {"node": np.asarray(node), "adj": np.asarray(adj),
              "dist": np.asarray(dist), "W_qkv": np.asarray(W_qkv),
              "W_out": np.asarray(W_out), "b_out": np.asarray(b_out)}
    out, _ = run(inputs, trace=False)
    return out



# revision 2
# speedup vs baseline: 1.0439x; 1.0439x over previous
"""Trainium2 Bass kernel for nn_Attention_21303037788751 (sparse_attention).

Reference computation (B=16, N=512, F=256, H=8, D=64):
    qkv  = node @ W_qkv                      -> q, k, v  [B,H,N,D]
    attn = softmax(q k^T / sqrt(D)) + 0.5*adj + 0.5*exp(-dist)
    out  = (attn @ v) reshaped  @ W_out + b_out

Sharding: data-parallel over batch, 2 batches per core on 8 NeuronCores.

v2 design (all matmul operands bf16, inputs pre-cast on host):
  - nodeT, adjT, distT loaded via DMA-transpose (XBAR) -> zero PE transposes.
  - G^T = adjT + exp(-distT) computed in the transposed layout directly.
  - The G-part of the output is folded through W_out on the host:
    wvw = Wv @ (0.5*W_out)  [F,F]; on-chip VW = node @ wvw  [N,F] and
    Y_G = G @ VW enters the output projection as 4 extra PSUM accumulations
    with lhsT = G^T chunks (replaces the old 32 OT2 matmuls of N=512).
  - Per-(head-pair, batch) software pipeline: ST -> exp(ACT) -> OT1, with
    qk/v/VW projections as PE filler so ACT overlaps from the start.
  - Softmax sums via the augmented-V stationary trick: even head's
    stationary is [v_e | 1] (sum lands at psum partition 64), odd head's is
    [1 | 0*63 | v_o] (sum at partition 0, outputs at 64..127).
"""

import sys

sys.path.insert(0, "/opt/trn_rl_repo")

import numpy as np

B, N, F = 16, 512, 256
H, D = 8, 64
INNER = H * D          # 512
NC_COUNT = 8
PB = B // NC_COUNT     # batches per core
P = 128
SCALE = D ** -0.5      # 0.125
VBLK = 193             # per-pair vaug block: [1 | 0*63 | v_o(64) | v_e(64) | 1]

_CACHE = {}


def _col_perm():
    """Column permutation of W_qkv: head-pair [q_h0|q_h1|k_h0|k_h1] blocks,
    then all v columns grouped by head."""
    order = []
    for p in range(H // 2):
        h0, h1 = 2 * p, 2 * p + 1
        order += [h0 * 192 + d for d in range(64)]
        order += [h1 * 192 + d for d in range(64)]
        order += [h0 * 192 + 64 + d for d in range(64)]
        order += [h1 * 192 + 64 + d for d in range(64)]
    for h in range(H):
        order += [h * 192 + 128 + d for d in range(64)]
    return np.array(order)


def build_program():
    import concourse.bass as bass
    import concourse.tile as tile
    from concourse import bacc, mybir

    f32 = mybir.dt.float32
    bf16 = mybir.dt.bfloat16
    EXP = mybir.ActivationFunctionType.Exp
    MULT = mybir.AluOpType.mult

    nc = bacc.Bacc("TRN2", target_bir_lowering=False, debug=False,
                   num_devices=NC_COUNT)

    node_d = nc.dram_tensor("node", [PB, N, F], bf16, kind="ExternalInput").ap()
    adj_d = nc.dram_tensor("adj", [PB, N, N], bf16, kind="ExternalInput").ap()
    dist_d = nc.dram_tensor("dist", [PB, N, N], bf16, kind="ExternalInput").ap()
    wqkv_d = nc.dram_tensor("wqkv", [F, 3 * INNER], bf16, kind="ExternalInput").ap()
    wout_d = nc.dram_tensor("wout", [INNER, F], bf16, kind="ExternalInput").ap()
    wvw_d = nc.dram_tensor("wvw", [F, F], bf16, kind="ExternalInput").ap()
    bout_d = nc.dram_tensor("bout", [1, F], bf16, kind="ExternalInput").ap()
    out_d = nc.dram_tensor("out", [PB, N, F], f32, kind="ExternalOutput").ap()

    with tile.TileContext(nc) as tc:
        with tc.tile_pool(name="const", bufs=1) as cpool, \
             tc.tile_pool(name="work", bufs=2) as wpool, \
             tc.tile_pool(name="qk", bufs=4) as qkpool, \
             tc.tile_pool(name="ex", bufs=4) as expool, \
             tc.tile_pool(name="big", bufs=3, space="PSUM") as psb, \
             tc.tile_pool(name="small", bufs=2, space="PSUM") as pss:

            S = [dict() for _ in range(PB)]

            # ---- weights: pair-0 qk columns first, then the rest ----------
            wqkv_r = cpool.tile([P, 2, 3 * INNER], bf16)
            wq_view = wqkv_d.rearrange("(kt p) m -> p kt m", p=P)
            nc.sync.dma_start(wqkv_r[:, :, 0:256], wq_view[:, :, 0:256])

            # ---- nodeT via DMA transpose ----------------------------------
            for b in range(PB):
                nt = wpool.tile([P, 2, N], bf16, tag="nodeT", name=f"nodeT_{b}")
                S[b]["nodeT"] = nt
                for kt in range(2):
                    nc.sync.dma_start(nt[:, kt, :],
                                      node_d[b][:, kt * P:(kt + 1) * P],
                                      transpose=True)

            nc.sync.dma_start(wqkv_r[:, :, 256:3 * INNER],
                              wq_view[:, :, 256:3 * INNER])
            wout_r = cpool.tile([P, 4, F], bf16)
            nc.sync.dma_start(wout_r[:], wout_d.rearrange("(kt p) f -> p kt f", p=P))
            wvw_r = cpool.tile([P, 2, F], bf16)
            nc.sync.dma_start(wvw_r[:], wvw_d.rearrange("(kt p) f -> p kt f", p=P))
            bout_r = cpool.tile([1, F], bf16)
            nc.sync.dma_start(bout_r[:], bout_d[:])

            # ---- G^T inputs via DMA transpose -----------------------------
            for b in range(PB):
                adjT = wpool.tile([P, 4, N], bf16, tag="adjT", name=f"adjT_{b}")
                distT = wpool.tile([P, 4, N], bf16, tag="distT",
                                   name=f"distT_{b}")
                S[b]["gt"] = adjT
                S[b]["distT"] = distT
                for jb in range(4):
                    nc.sync.dma_start(distT[:, jb, :],
                                      dist_d[b][:, jb * P:(jb + 1) * P],
                                      transpose=True)
                for jb in range(4):
                    nc.sync.dma_start(adjT[:, jb, :],
                                      adj_d[b][:, jb * P:(jb + 1) * P],
                                      transpose=True)

            # ---- constants ------------------------------------------------
            ones_row = cpool.tile([1, P], bf16)
            nc.gpsimd.memset(ones_row[:], 1.0)

            # vaug: [1 | 0*63 | v_o | v_e | 1] per (jb, pair)
            for b in range(PB):
                vaug = wpool.tile([P, 4, 4, VBLK], bf16, tag="vaug",
                                  name=f"vaug_{b}")
                S[b]["vaug"] = vaug
                nc.gpsimd.memset(vaug[:, :, :, 0:64], 0.0)
                nc.gpsimd.memset(vaug[:, :, :, 0:1], 1.0)
                nc.gpsimd.memset(vaug[:, :, :, 192:193], 1.0)

            def qk_proj(p, b):
                qk_ps = psb.tile([P, 2, N], f32, tag="big",
                                 name=f"qkps_{p}_{b}")
                base = p * 256
                nt = S[b]["nodeT"]
                for kt in range(2):
                    nc.tensor.matmul(qk_ps[:, 0, :],
                                     wqkv_r[:, kt, base:base + P],
                                     nt[:, kt, :],
                                     start=(kt == 0), stop=(kt == 1))
                for kt in range(2):
                    nc.tensor.matmul(qk_ps[:, 1, :],
                                     wqkv_r[:, kt, base + P:base + 256],
                                     nt[:, kt, :],
                                     start=(kt == 0), stop=(kt == 1))
                qk = qkpool.tile([P, 2, N], bf16, tag="qk",
                                 name=f"qk_{p}_{b}")
                nc.vector.tensor_copy(qk[:], qk_ps[:])
                S[b][f"qk{p}"] = qk

            def v_proj(b):
                vaug, nt = S[b]["vaug"], S[b]["nodeT"]
                for jh in range(2):
                    v_ps = psb.tile([P, 2, N], f32, tag="big",
                                    name=f"vps_{b}_{jh}")
                    for j in range(2):
                        jb = jh * 2 + j
                        for kt in range(2):
                            nc.tensor.matmul(
                                v_ps[:, j, :],
                                nt[:, kt, jb * P:(jb + 1) * P],
                                wqkv_r[:, kt, 2 * INNER:3 * INNER],
                                start=(kt == 0), stop=(kt == 1))
                    v4 = v_ps[:].rearrange("q two (pr par d) -> q two pr par d",
                                           par=2, d=64)
                    nc.vector.tensor_copy(vaug[:, jh * 2:jh * 2 + 2, :, 128:192],
                                          v4[:, :, :, 0, :])
                    nc.vector.tensor_copy(vaug[:, jh * 2:jh * 2 + 2, :, 64:128],
                                          v4[:, :, :, 1, :])

            def vw_proj(b):
                nt = S[b]["nodeT"]
                vw = wpool.tile([P, 4, F], bf16, tag="vw", name=f"vw_{b}")
                S[b]["vw"] = vw
                for jh in range(2):
                    vw_ps = pss.tile([P, 2, F], f32, tag="small",
                                     name=f"vwps_{b}_{jh}")
                    for j in range(2):
                        jb = jh * 2 + j
                        for kt in range(2):
                            nc.tensor.matmul(
                                vw_ps[:, j, :],
                                nt[:, kt, jb * P:(jb + 1) * P],
                                wvw_r[:, kt, :],
                                start=(kt == 0), stop=(kt == 1))
                    nc.vector.tensor_copy(vw[:, jh * 2:jh * 2 + 2, :], vw_ps[:])

            # otfin accumulators
            for b in range(PB):
                S[b]["otfin"] = wpool.tile([P, 4, N], bf16, tag="otfin",
                                           name=f"otfin_{b}")

            qk_proj(0, 0)
            qk_proj(0, 1)

            # ---- main attention pipeline ----------------------------------
            for p in range(H // 2):
                for b in range(PB):
                    s = S[b]
                    qq = s[f"qk{p}"][:, 0, :]
                    kk = s[f"qk{p}"][:, 1, :]
                    ex = [expool.tile([P, 4, N], bf16, tag="ex",
                                      name=f"ex_{b}_{2 * p}"),
                          expool.tile([P, 4, N], bf16, tag="ex",
                                      name=f"ex_{b}_{2 * p + 1}")]
                    s["ex"] = ex
                    for half in range(2):
                        st = [psb.tile([P, 2, N], f32, tag="big",
                                       name=f"st_e_{b}_{p}_{half}"),
                              psb.tile([P, 2, N], f32, tag="big",
                                       name=f"st_o_{b}_{p}_{half}")]
                        # even/odd heads on disjoint PE row strips -> the
                        # alternating matmuls execute concurrently
                        for j in range(2):
                            jb = half * 2 + j
                            for odd in range(2):
                                lo = odd * 64
                                nc.tensor.matmul(
                                    st[odd][:, j, :],
                                    kk[lo:lo + 64, jb * P:(jb + 1) * P],
                                    qq[lo:lo + 64, :],
                                    start=True, stop=True)
                        for odd in range(2):
                            nc.scalar.activation(
                                ex[odd][:, half * 2:half * 2 + 2, :],
                                st[odd][:], EXP, scale=SCALE)

                # PE fillers + ACT/Pool side-work for the G path
                if p == 0:
                    v_proj(0)
                    v_proj(1)
                    qk_proj(1, 0)
                    qk_proj(1, 1)
                    dt0 = S[0]["distT"]
                    nc.scalar.activation(dt0[:, 0:2, :], dt0[:, 0:2, :],
                                         EXP, scale=-1.0)
                    nc.scalar.activation(dt0[:, 2:4, :], dt0[:, 2:4, :],
                                         EXP, scale=-1.0)
                    nc.gpsimd.tensor_add(S[0]["gt"][:], S[0]["gt"][:], dt0[:])
                elif p == 1:
                    vw_proj(0)
                    vw_proj(1)
                    qk_proj(2, 0)
                    qk_proj(2, 1)
                    dt1 = S[1]["distT"]
                    nc.scalar.activation(dt1[:, 0:2, :], dt1[:, 0:2, :],
                                         EXP, scale=-1.0)
                    nc.scalar.activation(dt1[:, 2:4, :], dt1[:, 2:4, :],
                                         EXP, scale=-1.0)
                    nc.gpsimd.tensor_add(S[1]["gt"][:], S[1]["gt"][:], dt1[:])
                elif p == 2:
                    qk_proj(3, 0)
                    qk_proj(3, 1)

                for b in range(PB):
                    s = S[b]
                    vaug, ex = s["vaug"], s["ex"]
                    recbc = wpool.tile([P, N], f32, tag="recbc",
                                       name=f"recbc_{p}_{b}")
                    for odd in range(2):
                        ot1_ps = pss.tile([P, N], f32, tag="small",
                                          name=f"ot1_{b}_{p}_{odd}")
                        if not odd:
                            out_sl, av_sl, sm_sl = (slice(0, 65), slice(0, 64),
                                                    slice(64, 65))
                            rc_sl = slice(0, 65)
                        else:
                            out_sl, av_sl, sm_sl = (slice(0, P), slice(64, P),
                                                    slice(0, 1))
                            rc_sl = slice(0, 1)
                        for jb in range(4):
                            if not odd:
                                lhsT = vaug[:, jb, p, 128:VBLK]
                            else:
                                lhsT = vaug[:, jb, p, 0:128]
                            nc.tensor.matmul(
                                ot1_ps[out_sl, :], lhsT, ex[odd][:, jb, :],
                                start=(jb == 0), stop=(jb == 3))

                        rec = wpool.tile([P, N], f32, tag="rec")
                        # approx recip is broken at base partition 64; run it
                        # from partition 0 over the span (extra rows unused)
                        nc.vector.reciprocal_approx_fast(rec[rc_sl, :],
                                                         ot1_ps[rc_sl, :])
                        nc.gpsimd.dma_start(
                            recbc[av_sl, :],
                            rec[sm_sl, None, :].to_broadcast((1, 64, N)))
                        nc.vector.tensor_tensor(
                            s["otfin"][av_sl, p, :], ot1_ps[av_sl, :],
                            recbc[av_sl, :], MULT)

            # ---- output projection ----------------------------------------
            for b in range(PB):
                s = S[b]
                for nb in range(4):
                    y_ps = pss.tile([P, F], f32, tag="small",
                                    name=f"y_{b}_{nb}")
                    for kt in range(4):
                        nc.tensor.matmul(
                            y_ps[:], s["otfin"][:, kt, nb * P:(nb + 1) * P],
                            wout_r[:, kt, :],
                            start=(kt == 0), stop=False)
                    for kt in range(4):
                        nc.tensor.matmul(
                            y_ps[:], s["gt"][:, kt, nb * P:(nb + 1) * P],
                            s["vw"][:, kt, :],
                            start=False, stop=False)
                    nc.tensor.matmul(y_ps[:], ones_row[:], bout_r[:],
                                     start=False, stop=True)
                    y_sb = wpool.tile([P, F], f32, tag="y")
                    nc.scalar.copy(y_sb[:], y_ps[:])
                    nc.sync.dma_start(out_d[b][nb * P:(nb + 1) * P, :], y_sb[:])

    nc.compile()
    return nc


def _get_program():
    if "nc" not in _CACHE:
        _CACHE["nc"] = build_program()
    return _CACHE["nc"]


def run(inputs, trace=False):
    """Run on 8 cores; returns (full_output, BassKernelResults)."""
    import ml_dtypes
    from concourse.bass_utils import run_bass_kernel_spmd

    bf = ml_dtypes.bfloat16
    nc = _get_program()
    wqkv_f = np.asarray(inputs["W_qkv"], dtype=np.float32)
    wout_f = np.asarray(inputs["W_out"], dtype=np.float32)
    wqkv_p = np.ascontiguousarray(wqkv_f[:, _col_perm()]).astype(bf)
    wout = np.ascontiguousarray(wout_f).astype(bf)
    vcols = np.array([h * 192 + 128 + d for h in range(H) for d in range(D)])
    wvw = (wqkv_f[:, vcols] @ (0.5 * wout_f)).astype(bf)
    bout = np.asarray(inputs["b_out"], dtype=np.float32).reshape(1, F).astype(bf)
    node = np.asarray(inputs["node"], dtype=np.float32).astype(bf)
    adj = np.asarray(inputs["adj"], dtype=np.float32).astype(bf)
    dist = np.asarray(inputs["dist"], dtype=np.float32).astype(bf)

    in_maps = []
    for c in range(NC_COUNT):
        sl = slice(c * PB, (c + 1) * PB)
        in_maps.append({
            "node": np.ascontiguousarray(node[sl]),
            "adj": np.ascontiguousarray(adj[sl]),
            "dist": np.ascontiguousarray(dist[sl]),
            "wqkv": wqkv_p,
            "wout": wout,
            "wvw": wvw,
            "bout": bout,
        })
    res = run_bass_kernel_spmd(nc, in_maps, core_ids=list(range(NC_COUNT)),
                               trace=trace)
    out = np.concatenate([res.results[c]["out"] for c in range(NC_COUNT)], axis=0)
    return out, res


def kernel(node, adj, dist, node_mask, adj_mask, dist_mask, W_qkv, W_out, b_out):
    inputs = {"node": np.asarray(node), "adj": np.asarray(adj),
              "dist": np.asarray(dist), "W_qkv": np.asarray(W_qkv),
              "W_out": np.asarray(W_out), "b_out": np.asarray(b_out)}
    out, _ = run(inputs, trace=False)
    return out


# revision 7
# speedup vs baseline: 1.1881x; 1.1381x over previous
"""Trainium2 Bass kernel for nn_Attention_21303037788751 (sparse_attention).

Reference computation (B=16, N=512, F=256, H=8, D=64):
    qkv  = node @ W_qkv                      -> q, k, v  [B,H,N,D]
    attn = softmax(q k^T / sqrt(D)) + 0.5*adj + 0.5*exp(-dist)
    out  = (attn @ v) reshaped  @ W_out + b_out

Sharding: data-parallel over batch, 2 batches per core on 8 NeuronCores.

v3 design (all matmul operands bf16, inputs pre-cast on host):
  - interleaved row labels (n = q*4 + slot) throughout, as in the f32r
    baseline, so all stages use large contiguous DMAs; the output DMA
    un-permutes.
  - The G-part of the output is folded through W_out on the host:
    wvw = Wv @ (0.5*W_out)  [F,F]; on-chip VW = node @ wvw  [N,F] and
    Y_G = G @ VW enters the output projection as 4 extra PSUM
    accumulations with lhsT = G^T chunks (replaces 32 OT2 matmuls).
  - Software-pipelined PE stream: OT1 for pair p is emitted one pair late,
    interleaved in small "filler units" (v/vw/qk projections, G^T
    transposes, OT1 groups) between the ST half-groups of the next pair,
    keeping the PE dense so its p-state stays at full clock.
  - Softmax sums via the augmented-V stationary trick: even head's
    stationary is [v_e | 1] (sum lands at psum partition 64), odd head's
    is [1 | 0*63 | v_o] (sum at partition 0, outputs at 64..127).
"""

import sys

sys.path.insert(0, "/opt/trn_rl_repo")

import numpy as np

B, N, F = 16, 512, 256
H, D = 8, 64
INNER = H * D          # 512
NC_COUNT = 8
PB = B // NC_COUNT     # batches per core
P = 128
SCALE = D ** -0.5      # 0.125
VBLK = 193             # per-pair vaug block: [1 | 0*63 | v_o(64) | v_e(64) | 1]

_CACHE = {}


def _col_perm():
    """Column permutation of W_qkv: head-pair [q_h0|q_h1|k_h0|k_h1] blocks,
    then all v columns grouped by head."""
    order = []
    for p in range(H // 2):
        h0, h1 = 2 * p, 2 * p + 1
        order += [h0 * 192 + d for d in range(64)]
        order += [h1 * 192 + d for d in range(64)]
        order += [h0 * 192 + 64 + d for d in range(64)]
        order += [h1 * 192 + 64 + d for d in range(64)]
    for h in range(H):
        order += [h * 192 + 128 + d for d in range(64)]
    return np.array(order)


def build_program():
    import concourse.bass as bass
    import concourse.tile as tile
    from concourse import bacc, mybir
    from concourse.masks import make_identity

    f32 = mybir.dt.float32
    bf16 = mybir.dt.bfloat16
    EXP = mybir.ActivationFunctionType.Exp
    MULT = mybir.AluOpType.mult

    nc = bacc.Bacc("TRN2", target_bir_lowering=False, debug=False,
                   num_devices=NC_COUNT)

    node_d = nc.dram_tensor("node", [PB, N, F], bf16, kind="ExternalInput").ap()
    adj_d = nc.dram_tensor("adj", [PB, N, N], bf16, kind="ExternalInput").ap()
    dist_d = nc.dram_tensor("dist", [PB, N, N], bf16, kind="ExternalInput").ap()
    wqkv_d = nc.dram_tensor("wqkv", [F, 3 * INNER], bf16, kind="ExternalInput").ap()
    wout_d = nc.dram_tensor("wout", [INNER, F], bf16, kind="ExternalInput").ap()
    wvw_d = nc.dram_tensor("wvw", [F, F], bf16, kind="ExternalInput").ap()
    bout_d = nc.dram_tensor("bout", [1, F], bf16, kind="ExternalInput").ap()
    out_d = nc.dram_tensor("out", [PB, N, F], f32, kind="ExternalOutput").ap()

    with tile.TileContext(nc) as tc:
        with tc.tile_pool(name="const", bufs=1) as cpool, \
             tc.tile_pool(name="work", bufs=2) as wpool, \
             tc.tile_pool(name="qk", bufs=6) as qkpool, \
             tc.tile_pool(name="ex", bufs=8) as expool, \
             tc.tile_pool(name="pst", bufs=3, space="PSUM") as pst, \
             tc.tile_pool(name="psm", bufs=2, space="PSUM") as psm:

            S = [dict() for _ in range(PB)]

            # ---- input staging (sync queue order = priority) --------------
            wqkv_r = cpool.tile([P, 2, 3 * INNER], bf16)
            wq_view = wqkv_d.rearrange("(kt p) m -> p kt m", p=P)
            nc.sync.dma_start(wqkv_r[:, :, 0:256], wq_view[:, :, 0:256])
            for b in range(PB):
                node_t = wpool.tile([P, 4, F], bf16, tag="node",
                                    name=f"node_{b}")
                S[b]["node"] = node_t
                nc.sync.dma_start(node_t[:],
                                  node_d[b].rearrange("(p nb) f -> p nb f", nb=4))
            nc.sync.dma_start(wqkv_r[:, :, 256:3 * INNER],
                              wq_view[:, :, 256:3 * INNER])
            wout_r = cpool.tile([P, 4, F], bf16)
            nc.sync.dma_start(wout_r[:], wout_d.rearrange("(kt p) f -> p kt f", p=P))
            wvw_r = cpool.tile([P, 2, F], bf16)
            nc.sync.dma_start(wvw_r[:], wvw_d.rearrange("(kt p) f -> p kt f", p=P))
            bout_r = cpool.tile([1, F], bf16)
            nc.sync.dma_start(bout_r[:], bout_d[:])
            for b in range(PB):
                g_sb = wpool.tile([P, 4, N], bf16, tag="adj", name=f"adj_{b}")
                distt = wpool.tile([P, 4, N], bf16, tag="dist",
                                   name=f"dist_{b}")
                S[b]["g"] = g_sb
                S[b]["dist"] = distt
                nc.sync.dma_start(distt[:],
                                  dist_d[b].rearrange("(p ib) j -> p ib j", ib=4))
                nc.sync.dma_start(g_sb[:],
                                  adj_d[b].rearrange("(p ib) j -> p ib j", ib=4))

            # ---- constants (Pool) -----------------------------------------
            ident = cpool.tile([P, P], bf16)
            make_identity(nc, ident)
            ones_row = cpool.tile([1, P], bf16)
            nc.gpsimd.memset(ones_row[:], 1.0)
            for b in range(PB):
                vaug = wpool.tile([P, 4, 4, VBLK], bf16, tag="vaug",
                                  name=f"vaug_{b}")
                S[b]["vaug"] = vaug
                nc.gpsimd.memset(vaug[:, :, :, 0:64], 0.0)
                nc.gpsimd.memset(vaug[:, :, :, 0:1], 1.0)
                nc.gpsimd.memset(vaug[:, :, :, 192:193], 1.0)
                S[b]["otfin"] = wpool.tile([P, 4, N], bf16, tag="otfin",
                                           name=f"otfin_{b}")

            # ---- nodeT (PE transposes; starts the PE stream early) --------
            for b in range(PB):
                nodeT = wpool.tile([P, 2, N], bf16, tag="nodeT",
                                   name=f"nodeT_{b}")
                S[b]["nodeT"] = nodeT
                tr_ps = pst.tile([P, 2, N], bf16, tag="st", name=f"ntr_{b}")
                for kt in range(2):
                    for nb in range(4):
                        nc.tensor.transpose(
                            tr_ps[:, kt, nb * P:(nb + 1) * P],
                            S[b]["node"][:, nb, kt * P:(kt + 1) * P],
                            ident[:])
                nc.vector.tensor_copy(nodeT[:], tr_ps[:])

            # ---- filler units ---------------------------------------------
            def qk_unit(p, b):
                def emit():
                    qk_ps = pst.tile([P, 2, N], f32, tag="st",
                                     name=f"qkps_{p}_{b}")
                    base = p * 256
                    nt = S[b]["nodeT"]
                    for kt in range(2):
                        nc.tensor.matmul(qk_ps[:, 0, :],
                                         wqkv_r[:, kt, base:base + P],
                                         nt[:, kt, :],
                                         start=(kt == 0), stop=(kt == 1))
                    for kt in range(2):
                        nc.tensor.matmul(qk_ps[:, 1, :],
                                         wqkv_r[:, kt, base + P:base + 256],
                                         nt[:, kt, :],
                                         start=(kt == 0), stop=(kt == 1))
                    qk = qkpool.tile([P, 2, N], bf16, tag="qk",
                                     name=f"qk_{p}_{b}")
                    nc.vector.tensor_copy(qk[:], qk_ps[:])
                    S[b][f"qk{p}"] = qk
                return emit

            def v_unit(b, jh):
                def emit():
                    vaug, nt = S[b]["vaug"], S[b]["nodeT"]
                    v_ps = pst.tile([P, 2, N], f32, tag="st",
                                    name=f"vps_{b}_{jh}")
                    for j in range(2):
                        jb = jh * 2 + j
                        for kt in range(2):
                            nc.tensor.matmul(
                                v_ps[:, j, :],
                                nt[:, kt, jb * P:(jb + 1) * P],
                                wqkv_r[:, kt, 2 * INNER:3 * INNER],
                                start=(kt == 0), stop=(kt == 1))
                    v4 = v_ps[:].rearrange("q two (pr par d) -> q two pr par d",
                                           par=2, d=64)
                    nc.vector.tensor_copy(vaug[:, jh * 2:jh * 2 + 2, :, 128:192],
                                          v4[:, :, :, 0, :])
                    nc.vector.tensor_copy(vaug[:, jh * 2:jh * 2 + 2, :, 64:128],
                                          v4[:, :, :, 1, :])
                return emit

            def vw_unit(b, jh):
                def emit():
                    nt = S[b]["nodeT"]
                    if "vw" not in S[b]:
                        S[b]["vw"] = wpool.tile([P, 4, F], bf16, tag="vw",
                                                name=f"vw_{b}")
                    vw = S[b]["vw"]
                    vw_ps = psm.tile([P, 2, F], f32, tag="sm",
                                     name=f"vwps_{b}_{jh}")
                    for j in range(2):
                        jb = jh * 2 + j
                        for kt in range(2):
                            nc.tensor.matmul(
                                vw_ps[:, j, :],
                                nt[:, kt, jb * P:(jb + 1) * P],
                                wvw_r[:, kt, :],
                                start=(kt == 0), stop=(kt == 1))
                    nc.vector.tensor_copy(vw[:, jh * 2:jh * 2 + 2, :], vw_ps[:])
                return emit

            def gt_unit(b, jh):
                def emit():
                    if "gt" not in S[b]:
                        S[b]["gt"] = wpool.tile([P, 4, N], bf16, tag="gt",
                                                name=f"gt_{b}")
                    gt = S[b]["gt"]
                    g4 = S[b]["g"][:].rearrange("q ib (jq four) -> q ib jq four",
                                                four=4)
                    tr_ps = pst.tile([P, 2, N], bf16, tag="st",
                                     name=f"gtr_{b}_{jh}")
                    for j in range(2):
                        jb = jh * 2 + j
                        for ib in range(4):
                            nc.tensor.transpose(
                                tr_ps[:, j, ib * P:(ib + 1) * P],
                                g4[:, ib, :, jb],
                                ident[:])
                    nc.vector.tensor_copy(gt[:, jh * 2:jh * 2 + 2, :], tr_ps[:])
                return emit

            def ot1_unit(p, b, odd):
                def emit():
                    s = S[b]
                    vaug, ex = s["vaug"], s[f"ex{p}"]
                    if odd == 0:
                        s["recbc"] = wpool.tile([P, N], f32, tag="recbc",
                                                name=f"recbc_{p}_{b}")
                    recbc = s["recbc"]
                    ot1_ps = psm.tile([P, N], f32, tag="sm",
                                      name=f"ot1_{b}_{p}_{odd}")
                    if not odd:
                        out_sl, av_sl, sm_sl = (slice(0, 65), slice(0, 64),
                                                slice(64, 65))
                        rc_sl = slice(0, 65)
                    else:
                        out_sl, av_sl, sm_sl = (slice(0, P), slice(64, P),
                                                slice(0, 1))
                        rc_sl = slice(0, 1)
                    for jb in range(4):
                        if not odd:
                            lhsT = vaug[:, jb, p, 128:VBLK]
                        else:
                            lhsT = vaug[:, jb, p, 0:128]
                        nc.tensor.matmul(
                            ot1_ps[out_sl, :], lhsT, ex[odd][:, jb, :],
                            start=(jb == 0), stop=(jb == 3))
                    rec = wpool.tile([P, N], f32, tag="rec")
                    # approx recip is broken at base partition 64; run it
                    # from partition 0 over the span (extra rows unused)
                    nc.vector.reciprocal_approx_fast(rec[rc_sl, :],
                                                     ot1_ps[rc_sl, :])
                    nc.sync.dma_start(
                        recbc[av_sl, :],
                        rec[sm_sl, None, :].to_broadcast((1, 64, N)))
                    nc.vector.tensor_tensor(
                        s["otfin"][av_sl, p, :], ot1_ps[av_sl, :],
                        recbc[av_sl, :], MULT)
                return emit

            from collections import deque
            fillers = deque()

            def pop_fillers(k):
                for _ in range(k):
                    if fillers:
                        fillers.popleft()()

            # ---- pair 0 projections ---------------------------------------
            qk_unit(0, 0)()
            qk_unit(0, 1)()

            for b in range(PB):
                fillers.append(v_unit(b, 0))
                fillers.append(v_unit(b, 1))

            # ---- main attention pipeline ----------------------------------
            for p in range(H // 2):
                if p + 1 < H // 2:
                    fillers.appendleft(qk_unit(p + 1, 1))
                    fillers.appendleft(qk_unit(p + 1, 0))
                if p == 2:
                    fillers.append(gt_unit(0, 0))
                    fillers.append(gt_unit(0, 1))
                if p == 3:
                    fillers.append(gt_unit(1, 0))
                    fillers.append(gt_unit(1, 1))
                    for b in range(PB):
                        fillers.append(vw_unit(b, 0))
                        fillers.append(vw_unit(b, 1))
                for b in range(PB):
                    s = S[b]
                    qq = s[f"qk{p}"][:, 0, :]
                    kk = s[f"qk{p}"][:, 1, :]
                    ex = [expool.tile([P, 4, N], bf16, tag="ex",
                                      name=f"ex_{b}_{2 * p}"),
                          expool.tile([P, 4, N], bf16, tag="ex",
                                      name=f"ex_{b}_{2 * p + 1}")]
                    s[f"ex{p}"] = ex
                    for half in range(2):
                        st = [pst.tile([P, 2, N], f32, tag="st",
                                       name=f"st_e_{b}_{p}_{half}"),
                              pst.tile([P, 2, N], f32, tag="st",
                                       name=f"st_o_{b}_{p}_{half}")]
                        # even/odd heads on disjoint PE row strips -> the
                        # alternating matmuls execute concurrently
                        for j in range(2):
                            jb = half * 2 + j
                            for odd in range(2):
                                lo = odd * 64
                                nc.tensor.matmul(
                                    st[odd][:, j, :],
                                    kk[lo:lo + 64, jb * P:(jb + 1) * P],
                                    qq[lo:lo + 64, :],
                                    start=True, stop=True)
                        for odd in range(2):
                            nc.scalar.activation(
                                ex[odd][:, half * 2:half * 2 + 2, :],
                                st[odd][:], EXP, scale=SCALE)
                        pop_fillers(2)

                # ACT-side G work after this pair's exps
                if p == 0:
                    dt0 = S[0]["dist"]
                    nc.scalar.activation(dt0[:, 0:2, :], dt0[:, 0:2, :],
                                         EXP, scale=-1.0)
                    nc.scalar.activation(dt0[:, 2:4, :], dt0[:, 2:4, :],
                                         EXP, scale=-1.0)
                elif p == 1:
                    dt1 = S[1]["dist"]
                    nc.scalar.activation(dt1[:, 0:2, :], dt1[:, 0:2, :],
                                         EXP, scale=-1.0)
                    nc.scalar.activation(dt1[:, 2:4, :], dt1[:, 2:4, :],
                                         EXP, scale=-1.0)
                    g0, d0 = S[0]["g"], S[0]["dist"]
                    nc.gpsimd.tensor_add(g0[:, 0:2, :], g0[:, 0:2, :],
                                         d0[:, 0:2, :])
                    nc.gpsimd.tensor_add(g0[:, 2:4, :], g0[:, 2:4, :],
                                         d0[:, 2:4, :])
                elif p == 2:
                    g1, d1 = S[1]["g"], S[1]["dist"]
                    nc.gpsimd.tensor_add(g1[:, 0:2, :], g1[:, 0:2, :],
                                         d1[:, 0:2, :])
                    nc.gpsimd.tensor_add(g1[:, 2:4, :], g1[:, 2:4, :],
                                         d1[:, 2:4, :])

                # enqueue this pair's OT1 work (runs as next pair's filler)
                for b in range(PB):
                    fillers.append(ot1_unit(p, b, 0))
                    fillers.append(ot1_unit(p, b, 1))

            # flush remaining fillers (incl. OT1 for the last pair)
            while fillers:
                fillers.popleft()()

            # ---- output projection ----------------------------------------
            for b in range(PB):
                s = S[b]
                for nb in range(4):
                    y_ps = psm.tile([P, F], f32, tag="sm", name=f"y_{b}_{nb}")
                    for kt in range(4):
                        nc.tensor.matmul(
                            y_ps[:], s["otfin"][:, kt, nb * P:(nb + 1) * P],
                            wout_r[:, kt, :],
                            start=(kt == 0), stop=False)
                    for kt in range(4):
                        nc.tensor.matmul(
                            y_ps[:], s["gt"][:, kt, nb * P:(nb + 1) * P],
                            s["vw"][:, kt, :],
                            start=False, stop=False)
                    nc.tensor.matmul(y_ps[:], ones_row[:], bout_r[:],
                                     start=False, stop=True)
                    y_sb = wpool.tile([P, F], f32, tag="y")
                    nc.scalar.copy(y_sb[:], y_ps[:])
                    nc.sync.dma_start(
                        out_d[b].rearrange("(p four) f -> p four f", four=4)[:, nb, :],
                        y_sb[:])

    nc.compile()
    return nc


def _get_program():
    if "nc" not in _CACHE:
        _CACHE["nc"] = build_program()
    return _CACHE["nc"]


def run(inputs, trace=False):
    """Run on 8 cores; returns (full_output, BassKernelResults)."""
    import ml_dtypes
    from concourse.bass_utils import run_bass_kernel_spmd

    bf = ml_dtypes.bfloat16
    nc = _get_program()
    wqkv_f = np.asarray(inputs["W_qkv"], dtype=np.float32)
    wout_f = np.asarray(inputs["W_out"], dtype=np.float32)
    wqkv_p = np.ascontiguousarray(wqkv_f[:, _col_perm()]).astype(bf)
    wout = np.ascontiguousarray(wout_f).astype(bf)
    vcols = np.array([h * 192 + 128 + d for h in range(H) for d in range(D)])
    wvw = (wqkv_f[:, vcols] @ (0.5 * wout_f)).astype(bf)
    bout = np.asarray(inputs["b_out"], dtype=np.float32).reshape(1, F).astype(bf)
    node = np.asarray(inputs["node"], dtype=np.float32).astype(bf)
    adj = np.asarray(inputs["adj"], dtype=np.float32).astype(bf)
    dist = np.asarray(inputs["dist"], dtype=np.float32).astype(bf)

    in_maps = []
    for c in range(NC_COUNT):
        sl = slice(c * PB, (c + 1) * PB)
        in_maps.append({
            "node": np.ascontiguousarray(node[sl]),
            "adj": np.ascontiguousarray(adj[sl]),
            "dist": np.ascontiguousarray(dist[sl]),
            "wqkv": wqkv_p,
            "wout": wout,
            "wvw": wvw,
            "bout": bout,
        })
    res = run_bass_kernel_spmd(nc, in_maps, core_ids=list(range(NC_COUNT)),
                               trace=trace)
    out = np.concatenate([res.results[c]["out"] for c in range(NC_COUNT)], axis=0)
    return out, res


def kernel(node, adj, dist, node_mask, adj_mask, dist_mask, W_qkv, W_out, b_out):
    inputs = {"node": np.asarray(node), "adj": np.asarray(adj),
              "dist": np.asarray(dist), "W_qkv": np.asarray(W_qkv),
              "W_out": np.asarray(W_out), "b_out": np.asarray(b_out)}
    out, _ = run(inputs, trace=False)
    return out
